# revision 24
# baseline (speedup 1.0000x reference)
"""BERT self-attention kernel for Trainium2, sharded over 8 NeuronCores.

Problem: nn_CustomBertSelfAttention (B=2, S=2048, D=1024, H=16 heads, HD=64).

Sharding: tensor-parallel over heads. Core c owns heads {2c, 2c+1}, i.e.
columns [128c, 128c+128) of Wq/Wk/Wv and of the output. Every core reads the
full hidden_states (transposed + cast to bf16 on the host).

Design notes (v2 — scheduled for HAM-warm density and an ACT-bound steady
state; the baseline lost ~126us to a sticky cold PE p-state and ~40us to a
phase-serialized lead-in):

  * All PSUM pools coexist (no pool open/close phase serialization):
    pj (proj + transposes + swaps, 1x[128,1024]f32 ring = 2 banks),
    sps (scores, 2x[128,1024]f32 = 4 banks), cps (ctx accum, 1x[65,1024]f32
    = 2 banks). Attention lanes are processed serially per unit so a single
    ctx accumulator suffices.
  * Projections are emitted per (proj, batch, 1024-token chunk) chasing the
    x DMA; batch-0 q,k first so the first exp fires at ~17us. Batch-1
    projection work is emitted interleaved into early attention pairs as PE
    filler (the steady state is ACT-bound, PE has slack).
  * Scores are computed in kt-PAIRS as two K=64 row-tiles of the PE array
    (rows 0-63 / 64-127, auto tile_position from base partitions). The odd
    tile sources Q^T/K^T from half-swapped copies (q_swap/k_swap) built by
    SBUF->SBUF DMAs on the otherwise idle sync queue. The two-slot sps ring
    plus the 8-deep pt ring keep ScalarE's exp stream saturated.
  * ctx emission is gated on the vv copies being EMITTED first (the tile
    framework's dependency tracking is emission-ordered; a read emitted
    before its writer gets no RAW edge and races), and pt-ring slot reuse
    is gated on the old reader's emission likewise.
  * exp on ScalarE with the attention mask folded in as the activation's
    per-partition bias (exact: exp(s*sc + m)); V is augmented with a ones
    column ([V|1], 65-wide stationary) so row 64 of the ctx accumulator is
    the softmax denominator. No on-device normalization: the raw [65, S]
    goes to DRAM and the host divides (and adds the V bias).
  * A post-build IR pass drops InstLdweights whose stationary is identical
    to the one already loaded.
Host: out[u] = (ctx[0:64] / ctx[64])^T + bv  gathered into [B, S, D].
"""
import sys

sys.path.insert(0, "/opt/trn_rl_repo")

import numpy as np
import ml_dtypes

from concourse import bacc
import concourse.mybir as mybir
from concourse.tile import TileContext
from concourse.masks import make_identity
from concourse.bass_utils import run_bass_kernel_spmd

B, S, D, H, HD = 2, 2048, 1024, 16, 64
N_CORES = 8
HPC = H // N_CORES          # heads per core = 2
DC = D // N_CORES           # output/weight columns per core = 128
BS = B * S                  # 4096
NU = B * HPC                # attention units per core = 4
P = 128
F32 = mybir.dt.float32
BF16 = mybir.dt.bfloat16
KT = S // P                 # 16 key tiles per unit
QH = 1024                   # query lane width
NL = S // QH                # 2 query lanes per unit
CH = 1024                   # projection token chunk
NC_CH = S // CH             # chunks per batch = 2
W65 = HD + 1                # V_aug width (V columns + ones column)
W130 = 2 * W65              # two heads interleaved per key tile in vv[b]
DT = D // P                 # 8 contraction tiles
SCALE = float(1.0 / np.sqrt(HD))

DEDUPE_LDWEIGHTS = True
FILLER_INTERLEAVE = True

_cached_nc = None
DBG = {}  # emission-name registry for offline dependency audits


def _reg(kind, key, r):
    try:
        DBG.setdefault(kind, {})[key] = getattr(r, "ins", r).name
    except Exception:
        pass


def _ap_key(arg):
    """Stable identity key for an LDWEIGHTS stationary access pattern."""
    try:
        bass_ap = getattr(arg, "bass_ap", None)
        if bass_ap is not None:
            return ("bap", bass_ap.tensor.name, bass_ap.offset,
                    tuple(map(tuple, bass_ap.ap)), str(arg.dtype))
        return ("raw", getattr(arg, "memref", ""), arg.offset,
                tuple(map(tuple, arg.ap)), str(arg.dtype))
    except Exception:
        return ("repr", repr(arg))


def _dedupe_ldweights(nc, keep_mm_names=()):
    """Drop PE weight reloads whose stationary is already in the array.

    The tile legalizer splits every InstMatmult into InstLdweights +
    InstMatmult. Runs of matmuls that share a stationary reload it
    redundantly; the PE array retains the stationary across matmuls, so
    duplicate loads are pure overhead (~100ns each). Dependencies carried
    by a dropped load are merged into the next PE instruction so no
    synchronization is lost. Operates on the post-scheduler order, so only
    loads that are genuinely redundant at execution time are removed.
    """
    keep = set(keep_mm_names)
    pe = mybir.EngineType.PE
    for f in nc.m.functions:
        for blk in f.blocks:
            insts = blk.instructions
            pe_seq = [i for i in insts
                      if getattr(i, "engine", None) == pe]
            next_mm_name = {}
            pending = []
            for i in pe_seq:
                if type(i).__name__ == "InstLdweights":
                    pending.append(i)
                else:
                    for ld in pending:
                        next_mm_name[id(ld)] = i.name
                    pending = []
            drop = set()
            cur_key = None
            pending_merge = []
            for i in pe_seq:
                tn = type(i).__name__
                if tn == "InstLdweights":
                    key = (
                        _ap_key(i.ins[0]),
                        getattr(i, "is_transpose", None),
                        getattr(i, "perf_mode", None),
                        getattr(i, "tile_position", None),
                    )
                    if key == cur_key and next_mm_name.get(id(i)) not in keep:
                        drop.add(id(i))
                        pending_merge.append(i)
                    else:
                        cur_key = key
                elif pending_merge:
                    for ld in pending_merge:
                        i.merge_dependencies_from(ld)
                    pending_merge = []
            if drop:
                blk.instructions = [i for i in insts if id(i) not in drop]


def build_nc():
    nc = bacc.Bacc(None, target_bir_lowering=False)

    xT = nc.dram_tensor("xT", [D, BS], BF16, kind="ExternalInput")
    # weights host-pre-tiled to [P, DT*DC] so the DMA is a plain 2D copy
    w_in = {
        pr: nc.dram_tensor(f"w{pr}", [P, DT * DC], BF16, kind="ExternalInput")
        for pr in "qkv"
    }
    bqkv = nc.dram_tensor("bqkv", [DC, 3], F32, kind="ExternalInput")
    # mask host-pre-tiled to [P, B*KT] (key position on partitions)
    mkT = nc.dram_tensor("mkT", [P, B * KT], F32, kind="ExternalInput")
    out = nc.dram_tensor("out", [NU, W65, S], F32, kind="ExternalOutput")

    from contextlib import ExitStack

    with TileContext(nc) as tc, ExitStack() as es:
        const = es.enter_context(tc.tile_pool(name="const", bufs=1))
        wp = es.enter_context(tc.tile_pool(name="wsb", bufs=1))
        xp = es.enter_context(tc.tile_pool(name="xsb", bufs=1))
        qk = es.enter_context(tc.tile_pool(name="qksb", bufs=1))
        PT_RING = 12
        ptp = es.enter_context(tc.tile_pool(name="pt", bufs=PT_RING))
        obp = es.enter_context(tc.tile_pool(name="ob", bufs=2))
        # PSUM: all three pools coexist (2 + 4 + 2 = 8 banks)
        pj = es.enter_context(tc.tile_pool(name="pj", bufs=1, space="PSUM"))
        sp = es.enter_context(tc.tile_pool(name="sps", bufs=2, space="PSUM"))
        cp = es.enter_context(tc.tile_pool(name="cps", bufs=1, space="PSUM"))

        ident = const.tile([P, P], BF16)
        make_identity(nc, ident)
        b_sb = const.tile([DC, 3], F32)
        mk = const.tile([P, B * KT], F32)
        w_sb = {
            pr: wp.tile([P, DT * DC], BF16, tag=f"w{pr}", name=f"w{pr}sb")
            for pr in "qkv"
        }
        # x^T staged in SBUF: one tile per (d-tile, batch)
        xx = {
            (dt, b): xp.tile([P, S], BF16, tag=f"x{dt}_{b}", name=f"x{dt}_{b}")
            for b in range(B) for dt in range(DT)
        }

        # DMA order matters: the sync queue serializes transfers, so emit
        # exactly what the critical path needs first.
        nc.sync.dma_start(w_sb["q"][:], w_in["q"][:])
        nc.sync.dma_start(w_sb["k"][:], w_in["k"][:])
        for dt in range(DT):
            nc.sync.dma_start(xx[(dt, 0)][:], xT[dt * P:(dt + 1) * P, 0:S])
        nc.sync.dma_start(mk[:], mkT[:])
        nc.sync.dma_start(b_sb[:], bqkv[:])
        # (wv and batch-1 x DMAs are emitted after the chunk-0 swap DMAs so
        # the sync queue serves the first scores pair first)

        # Persistent per-core activations.  q_sb/k_sb: [dq, tokens] with
        # head h on partitions [64h, 64h+64).  q_swap/k_swap: partition
        # halves swapped, so each head is also available in the *other*
        # half of the PE array for the kt-pair row-tiled scores.
        q_sb = [qk.tile([P, S], BF16, tag=f"qs{b}", name=f"qs{b}")
                for b in range(B)]
        k_sb = [qk.tile([P, S], BF16, tag=f"ks{b}", name=f"ks{b}")
                for b in range(B)]
        q_sw = [qk.tile([P, S], BF16, tag=f"qw{b}", name=f"qw{b}")
                for b in range(B)]
        k_sw = [qk.tile([P, S], BF16, tag=f"kw{b}", name=f"kw{b}")
                for b in range(B)]
        v_t = [qk.tile([P, S], BF16, tag=f"vt{b}", name=f"vt{b}")
               for b in range(B)]
        vv = [qk.tile([P, KT * W130], BF16, tag=f"vv{b}", name=f"vv{b}")
              for b in range(B)]

        # ones columns of the V_aug layout, written as plain 2D slices so
        # the dependency tracker reliably orders them against ctx reads
        for b in range(B):
            for kt in range(KT):
                for g in range(2):
                    col = kt * W130 + g * W65 + HD
                    _reg("ones", (b, kt, g),
                         nc.vector.memset(vv[b][:, col:col + 1], 1.0))

        vv_ready = [0, 0]   # kts of vv[b] whose copies have been emitted

        # ---------------- projection building blocks ----------------
        BCOL = {"q": 0, "k": 1}

        def evict_proj(pr, b, c, ps):
            c0 = c * CH
            if pr == "v":
                nc.vector.tensor_copy(v_t[b][:, c0:c0 + CH], ps[:])
            else:
                dst = q_sb[b] if pr == "q" else k_sb[b]
                j = BCOL[pr]
                nc.vector.tensor_scalar_add(
                    dst[:, c0:c0 + CH], ps[:], b_sb[:, j:j + 1])

        def proj_granules(pr, b, c):
            """One [128,1024] proj chunk as 4 thunks of 2 dt-steps each."""
            state = {}
            c0 = c * CH

            def gran(g, pr=pr, b=b, c=c, c0=c0, state=state):
                if g == 0:
                    state["ps"] = pj.tile([P, CH], F32, tag="pp", name="pp")
                ps = state["ps"]
                for dt in (2 * g, 2 * g + 1):
                    w = w_sb[pr][:, dt * DC:(dt + 1) * DC]
                    x = xx[(dt, b)]
                    nc.tensor.matmul(ps[:, 0:512], w, x[:, c0:c0 + 512],
                                     start=(dt == 0), stop=(dt == DT - 1))
                    nc.tensor.matmul(ps[:, 512:CH], w, x[:, c0 + 512:c0 + CH],
                                     start=(dt == 0), stop=(dt == DT - 1))
                if g == 3:
                    evict_proj(pr, b, c, ps)
            return [lambda g=g: gran(g) for g in range(4)]

        def emit_swap(pr, b, c):
            """Half-swapped q/k chunk via two SBUF->SBUF DMAs (partition
            re-basing is free on the DMA engines; the sync queue is idle
            mid-attention)."""
            srct = q_sb[b] if pr == "q" else k_sb[b]
            dst = q_sw[b] if pr == "q" else k_sw[b]
            c0 = c * CH
            nc.sync.dma_start(dst[0:HD, c0:c0 + CH],
                              srct[HD:P, c0:c0 + CH])
            nc.sync.dma_start(dst[HD:P, c0:c0 + CH],
                              srct[0:HD, c0:c0 + CH])

        def transp_granules(b, c):
            """8 V^T key-tile transposes into the vv layout, as 2 thunks.

            tp is bf16 [P, 2*CH] so its byte size matches the f32 [P, CH]
            tiles sharing the "pp" slot; only the first CH columns are used.
            """
            state = {}

            def gran(g, b=b, c=c, state=state):
                if g == 0:
                    state["tp"] = pj.tile([P, 2 * CH], BF16, tag="pp",
                                          name="tp")
                tp = state["tp"]
                for j in range(4 * g, 4 * g + 4):
                    kt = c * 8 + j
                    nc.tensor.transpose(
                        tp[:, j * P:(j + 1) * P],
                        v_t[b][:, kt * P:(kt + 1) * P],
                        ident[:],
                    )
                # plain 2D copies per (tile, head): robustly tracked deps
                for j in range(4 * g, 4 * g + 4):
                    kt = c * 8 + j
                    for gh in range(2):
                        r = nc.vector.tensor_copy(
                            vv[b][:, kt * W130 + gh * W65:
                                  kt * W130 + gh * W65 + HD],
                            tp[:, j * P + gh * HD:j * P + gh * HD + HD],
                        )
                        _reg("vvw", (b, kt, gh), r)
                # emission-order progress marker: ctx for these key tiles
                # may only be EMITTED after this point (the dependency
                # tracker is emission-ordered; a read emitted before its
                # writer gets no RAW edge and races)
                vv_ready[b] = max(vv_ready[b], c * 8 + 4 * (g + 1))
            return [lambda g=g: gran(g) for g in range(2)]

        # ---- batch-0 critical path: q-c0/k-c0/k-c1 chase the x DMA ----
        psq0 = sp.tile([P, CH], F32, tag="sps", name="sps")
        psk0 = sp.tile([P, CH], F32, tag="sps", name="sps")
        psk1 = pj.tile([P, CH], F32, tag="pp", name="pp")
        for dt in range(DT):
            wq = w_sb["q"][:, dt * DC:(dt + 1) * DC]
            wk = w_sb["k"][:, dt * DC:(dt + 1) * DC]
            x = xx[(dt, 0)]
            st, fin = (dt == 0), (dt == DT - 1)
            nc.tensor.matmul(psq0[:, 0:512], wq, x[:, 0:512],
                             start=st, stop=fin)
            nc.tensor.matmul(psq0[:, 512:CH], wq, x[:, 512:CH],
                             start=st, stop=fin)
            nc.tensor.matmul(psk0[:, 0:512], wk, x[:, 0:512],
                             start=st, stop=fin)
            nc.tensor.matmul(psk0[:, 512:CH], wk, x[:, 512:CH],
                             start=st, stop=fin)
            nc.tensor.matmul(psk1[:, 0:512], wk, x[:, CH:CH + 512],
                             start=st, stop=fin)
            nc.tensor.matmul(psk1[:, 512:CH], wk, x[:, CH + 512:S],
                             start=st, stop=fin)
        evict_proj("q", 0, 0, psq0)
        evict_proj("k", 0, 0, psk0)
        evict_proj("k", 0, 1, psk1)
        # chunk-0 swaps feed the first scores pair
        emit_swap("k", 0, 0)
        emit_swap("q", 0, 0)
        nc.sync.dma_start(w_sb["v"][:], w_in["v"][:])
        for dt in range(DT):
            nc.sync.dma_start(xx[(dt, 1)][:], xT[dt * P:(dt + 1) * P, S:BS])

        # ---- deferred work queue, paced into the attention pair loop ----
        G = []
        G += [lambda: emit_swap("k", 0, 1)]
        G += proj_granules("v", 0, 0)
        G += transp_granules(0, 0)
        G += proj_granules("q", 0, 1)
        G += [lambda: emit_swap("q", 0, 1)]
        G += proj_granules("v", 0, 1)
        G += transp_granules(0, 1)
        for c in range(NC_CH):
            G += proj_granules("q", 1, c)
            G += [lambda c=c: emit_swap("q", 1, c)]
            G += proj_granules("k", 1, c)
            G += [lambda c=c: emit_swap("k", 1, c)]
            G += proj_granules("v", 1, c)
            G += transp_granules(1, c)
        fi = 0
        if not FILLER_INTERLEAVE:
            for item in G:
                item()
            fi = len(G)

        # ---------------- attention ----------------
        with nc.named_scope("attn"):
            n_emitted_pairs = 0
            for u in range(NU):
                b, h = u // HPC, u % HPC
                # head h's rows in the natural / swapped layouts
                nat = slice(h * HD, (h + 1) * HD)
                swp = slice((1 - h) * HD, (2 - h) * HD)
                for l in range(NL):
                    q0 = l * QH
                    cps = cp.tile([W65, QH], F32, tag="cps", name="cps")
                    pts = [None] * KT

                    def emit_ctx(kt, b=b, h=h, cps=cps, pts=pts):
                        o0 = kt * W130 + h * W65
                        va = vv[b][:, o0:o0 + W65]
                        _reg("ctx", (u, l, kt, 0), nc.tensor.matmul(
                            cps[:, 0:512], va, pts[kt][:, 0:512],
                            start=(kt == 0), stop=(kt == KT - 1)))
                        _reg("ctx", (u, l, kt, 1), nc.tensor.matmul(
                            cps[:, 512:QH], va, pts[kt][:, 512:QH],
                            start=(kt == 0), stop=(kt == KT - 1)))

                    pending = []  # kts with pts awaiting ctx emission

                    def drain_until_vv(kt, b=b):
                        nonlocal fi
                        while vv_ready[b] <= kt and fi < len(G):
                            G[fi]()
                            fi += 1
                        assert vv_ready[b] > kt, "granule queue exhausted"

                    def flush_ctx(force_upto=None):
                        while pending and (
                                pending[0] < vv_ready[b]
                                or (force_upto is not None
                                    and pending[0] <= force_upto)):
                            kt = pending.pop(0)
                            if kt >= vv_ready[b]:
                                drain_until_vv(kt)
                            emit_ctx(kt)

                    for p in range(KT // 2):
                        ktE, ktO = 2 * p, 2 * p + 1
                        # pt-ring safety: slots reused by this pair's exps
                        # must have their reader ctx emitted first
                        flush_ctx(force_upto=ktO - PT_RING)
                        sE = sp.tile([P, QH], F32, tag="sps", name="sps")
                        sO = sp.tile([P, QH], F32, tag="sps", name="sps")
                        kE = k_sb[b][nat, ktE * P:(ktE + 1) * P]
                        kO = k_sw[b][swp, ktO * P:(ktO + 1) * P]
                        qE = q_sb[b][nat, q0:q0 + QH]
                        qO = q_sw[b][swp, q0:q0 + QH]
                        # interleave E/O so the two K=64 row-tiles can run
                        # concurrently when the PE is the constraint
                        nc.tensor.matmul(sE[:, 0:512], kE, qE[:, 0:512],
                                         start=True, stop=True)
                        nc.tensor.matmul(sO[:, 0:512], kO, qO[:, 0:512],
                                         start=True, stop=True)
                        nc.tensor.matmul(sE[:, 512:QH], kE, qE[:, 512:QH],
                                         start=True, stop=True)
                        nc.tensor.matmul(sO[:, 512:QH], kO, qO[:, 512:QH],
                                         start=True, stop=True)
                        flush_ctx()
                        for kt, s in ((ktE, sE), (ktO, sO)):
                            pt = ptp.tile([P, QH], BF16, tag="pt")
                            r = nc.scalar.activation(
                                pt[:], s[:],
                                mybir.ActivationFunctionType.Exp,
                                bias=mk[:, b * KT + kt:b * KT + kt + 1],
                                scale=SCALE,
                            )
                            _reg("exp", (u, l, kt), r)
                            pts[kt] = pt
                        pending += [ktE, ktO]
                        # deferred proj/layout granules ride the PE slack
                        n_emitted_pairs += 1
                        budget = 3 if n_emitted_pairs <= 8 else 2
                        while budget > 0 and fi < len(G):
                            G[fi]()
                            fi += 1
                            budget -= 1
                    flush_ctx(force_upto=KT - 1)
                    ob = obp.tile([W65, QH], F32, tag="ob")
                    nc.vector.tensor_copy(ob[:], cps[:])
                    nc.sync.dma_start(out[u, :, q0:q0 + QH], ob[:])
            while fi < len(G):
                G[fi]()
                fi += 1

    if DEDUPE_LDWEIGHTS:
        _dedupe_ldweights(nc)
    nc.compile()
    return nc


def _prep_in_maps(hidden_states, attention_mask, Wq, bq, Wk, bk, Wv, bv):
    bf = ml_dtypes.bfloat16
    hs = np.asarray(hidden_states, dtype=np.float32).reshape(BS, D)
    xT = np.ascontiguousarray(hs.T).astype(bf)
    # mask pre-tiled: mkT[p, b*KT + t] = mask[b, t*P + p]
    mkT = np.ascontiguousarray(
        np.asarray(attention_mask, dtype=np.float32).reshape(B, KT, P)
        .transpose(2, 0, 1).reshape(P, B * KT)
    )
    Ws = {"q": np.asarray(Wq, np.float32), "k": np.asarray(Wk, np.float32),
          "v": np.asarray(Wv, np.float32)}
    bs = {"q": np.asarray(bq, np.float32), "k": np.asarray(bk, np.float32),
          "v": np.asarray(bv, np.float32)}
    in_maps = []
    for c in range(N_CORES):
        sl = slice(c * DC, (c + 1) * DC)
        m = {"xT": xT, "mkT": mkT}
        for pr in "qkv":
            # pre-tiled: [P, DT*DC], column block dt = rows [dt*P,(dt+1)*P)
            wc = Ws[pr][:, sl].reshape(DT, P, DC).transpose(1, 0, 2)
            m[f"w{pr}"] = np.ascontiguousarray(wc.reshape(P, DT * DC)).astype(bf)
        m["bqkv"] = np.ascontiguousarray(
            np.stack([bs["q"][sl], bs["k"][sl], bs["v"][sl]], axis=1)
        )
        in_maps.append(m)
    return in_maps


def _gather(results, bv):
    bv = np.asarray(bv, np.float32)
    full = np.empty((B, S, D), dtype=np.float32)
    for c in range(N_CORES):
        o = results[c]["out"]  # [NU, 65, S] unnormalized ctx^T + denom row
        for b in range(B):
            for hl in range(HPC):
                u = b * HPC + hl
                col = c * DC + hl * HD
                ctx = o[u, :HD, :] / o[u, HD:HD + 1, :]
                full[b, :, col:col + HD] = ctx.T + bv[col:col + HD]
    return full


def kernel(hidden_states, attention_mask, Wq, bq, Wk, bk, Wv, bv, **run_kwargs):
    global _cached_nc
    if _cached_nc is None:
        _cached_nc = build_nc()
    in_maps = _prep_in_maps(
        hidden_states, attention_mask, Wq, bq, Wk, bk, Wv, bv
    )
    res = run_bass_kernel_spmd(
        _cached_nc, in_maps, core_ids=list(range(N_CORES)), **run_kwargs
    )
    full = _gather(res.results, bv)
    if run_kwargs:
        kernel.last_result = res
    return full


# revision 27
# speedup vs baseline: 1.0074x; 1.0074x over previous
"""BERT self-attention kernel for Trainium2, sharded over 8 NeuronCores.

Problem: nn_CustomBertSelfAttention (B=2, S=2048, D=1024, H=16 heads, HD=64).

Sharding: tensor-parallel over heads. Core c owns heads {2c, 2c+1}, i.e.
columns [128c, 128c+128) of Wq/Wk/Wv and of the output. Every core reads the
full hidden_states (transposed + cast to bf16 on the host).

Design notes (v2 — scheduled for HAM-warm density and an ACT-bound steady
state; the baseline lost ~126us to a sticky cold PE p-state and ~40us to a
phase-serialized lead-in):

  * All PSUM pools coexist (no pool open/close phase serialization):
    pj (proj + transposes + swaps, 1x[128,1024]f32 ring = 2 banks),
    sps (scores, 2x[128,1024]f32 = 4 banks), cps (ctx accum, 1x[65,1024]f32
    = 2 banks). Attention lanes are processed serially per unit so a single
    ctx accumulator suffices.
  * Projections are emitted per (proj, batch, 1024-token chunk) chasing the
    x DMA; batch-0 q,k first so the first exp fires at ~17us. Batch-1
    projection work is emitted interleaved into early attention pairs as PE
    filler (the steady state is ACT-bound, PE has slack).
  * Scores are computed in kt-PAIRS as two K=64 row-tiles of the PE array
    (rows 0-63 / 64-127, auto tile_position from base partitions). The odd
    tile sources Q^T/K^T from half-swapped copies (q_swap/k_swap) built by
    SBUF->SBUF DMAs on the otherwise idle sync queue. The two-slot sps ring
    plus the 8-deep pt ring keep ScalarE's exp stream saturated.
  * ctx emission is gated on the vv copies being EMITTED first (the tile
    framework's dependency tracking is emission-ordered; a read emitted
    before its writer gets no RAW edge and races), and pt-ring slot reuse
    is gated on the old reader's emission likewise.
  * exp on ScalarE with the attention mask folded in as the activation's
    per-partition bias (exact: exp(s*sc + m)); V is augmented with a ones
    column ([V|1], 65-wide stationary) so row 64 of the ctx accumulator is
    the softmax denominator. No on-device normalization: the raw [65, S]
    goes to DRAM and the host divides (and adds the V bias).
  * A post-build IR pass drops InstLdweights whose stationary is identical
    to the one already loaded.
Host: out[u] = (ctx[0:64] / ctx[64])^T + bv  gathered into [B, S, D].
"""
import sys

sys.path.insert(0, "/opt/trn_rl_repo")

import numpy as np
import ml_dtypes

from concourse import bacc
import concourse.mybir as mybir
from concourse.tile import TileContext
from concourse.masks import make_identity
from concourse.bass_utils import run_bass_kernel_spmd

B, S, D, H, HD = 2, 2048, 1024, 16, 64
N_CORES = 8
HPC = H // N_CORES          # heads per core = 2
DC = D // N_CORES           # output/weight columns per core = 128
BS = B * S                  # 4096
NU = B * HPC                # attention units per core = 4
P = 128
F32 = mybir.dt.float32
BF16 = mybir.dt.bfloat16
KT = S // P                 # 16 key tiles per unit
QH = 1024                   # query lane width
NL = S // QH                # 2 query lanes per unit
CH = 1024                   # projection token chunk
NC_CH = S // CH             # chunks per batch = 2
W65 = HD + 1                # V_aug width (V columns + ones column)
W130 = 2 * W65              # two heads interleaved per key tile in vv[b]
DT = D // P                 # 8 contraction tiles
SCALE = float(1.0 / np.sqrt(HD))

DEDUPE_LDWEIGHTS = True
FILLER_INTERLEAVE = True

_cached_nc = None
DBG = {}  # emission-name registry for offline dependency audits


def _reg(kind, key, r):
    try:
        DBG.setdefault(kind, {})[key] = getattr(r, "ins", r).name
    except Exception:
        pass


def _ap_key(arg):
    """Stable identity key for an LDWEIGHTS stationary access pattern."""
    try:
        bass_ap = getattr(arg, "bass_ap", None)
        if bass_ap is not None:
            return ("bap", bass_ap.tensor.name, bass_ap.offset,
                    tuple(map(tuple, bass_ap.ap)), str(arg.dtype))
        return ("raw", getattr(arg, "memref", ""), arg.offset,
                tuple(map(tuple, arg.ap)), str(arg.dtype))
    except Exception:
        return ("repr", repr(arg))


def _dedupe_ldweights(nc, keep_mm_names=()):
    """Drop PE weight reloads whose stationary is already in the array.

    The tile legalizer splits every InstMatmult into InstLdweights +
    InstMatmult. Runs of matmuls that share a stationary reload it
    redundantly; the PE array retains the stationary across matmuls, so
    duplicate loads are pure overhead (~100ns each). Dependencies carried
    by a dropped load are merged into the next PE instruction so no
    synchronization is lost. Operates on the post-scheduler order, so only
    loads that are genuinely redundant at execution time are removed.
    """
    keep = set(keep_mm_names)
    pe = mybir.EngineType.PE
    for f in nc.m.functions:
        for blk in f.blocks:
            insts = blk.instructions
            pe_seq = [i for i in insts
                      if getattr(i, "engine", None) == pe]
            next_mm_name = {}
            pending = []
            for i in pe_seq:
                if type(i).__name__ == "InstLdweights":
                    pending.append(i)
                else:
                    for ld in pending:
                        next_mm_name[id(ld)] = i.name
                    pending = []
            drop = set()
            cur_key = None
            pending_merge = []
            for i in pe_seq:
                tn = type(i).__name__
                if tn == "InstLdweights":
                    key = (
                        _ap_key(i.ins[0]),
                        getattr(i, "is_transpose", None),
                        getattr(i, "perf_mode", None),
                        getattr(i, "tile_position", None),
                    )
                    if key == cur_key and next_mm_name.get(id(i)) not in keep:
                        drop.add(id(i))
                        pending_merge.append(i)
                    else:
                        cur_key = key
                elif pending_merge:
                    for ld in pending_merge:
                        i.merge_dependencies_from(ld)
                    pending_merge = []
            if drop:
                blk.instructions = [i for i in insts if id(i) not in drop]


def build_nc():
    nc = bacc.Bacc(None, target_bir_lowering=False)

    xT = nc.dram_tensor("xT", [D, BS], BF16, kind="ExternalInput")
    # weights host-pre-tiled to [P, DT*DC] so the DMA is a plain 2D copy
    w_in = {
        pr: nc.dram_tensor(f"w{pr}", [P, DT * DC], BF16, kind="ExternalInput")
        for pr in "qkv"
    }
    bqkv = nc.dram_tensor("bqkv", [DC, 3], F32, kind="ExternalInput")
    # mask host-pre-tiled to [P, B*KT] (key position on partitions)
    mkT = nc.dram_tensor("mkT", [P, B * KT], F32, kind="ExternalInput")
    out = nc.dram_tensor("out", [NU, W65, S], F32, kind="ExternalOutput")

    from contextlib import ExitStack

    with TileContext(nc) as tc, ExitStack() as es:
        const = es.enter_context(tc.tile_pool(name="const", bufs=1))
        wp = es.enter_context(tc.tile_pool(name="wsb", bufs=1))
        xp = es.enter_context(tc.tile_pool(name="xsb", bufs=1))
        qk = es.enter_context(tc.tile_pool(name="qksb", bufs=1))
        ptp = es.enter_context(tc.tile_pool(name="pt", bufs=8))
        obp = es.enter_context(tc.tile_pool(name="ob", bufs=2))
        # PSUM: all three pools coexist (2 + 4 + 2 = 8 banks)
        pj = es.enter_context(tc.tile_pool(name="pj", bufs=1, space="PSUM"))
        sp = es.enter_context(tc.tile_pool(name="sps", bufs=2, space="PSUM"))
        cp = es.enter_context(tc.tile_pool(name="cps", bufs=1, space="PSUM"))

        ident = const.tile([P, P], BF16)
        make_identity(nc, ident)
        b_sb = const.tile([DC, 3], F32)
        mk = const.tile([P, B * KT], F32)
        w_sb = {
            pr: wp.tile([P, DT * DC], BF16, tag=f"w{pr}", name=f"w{pr}sb")
            for pr in "qkv"
        }
        # x^T staged in SBUF: one tile per (d-tile, batch)
        xx = {
            (dt, b): xp.tile([P, S], BF16, tag=f"x{dt}_{b}", name=f"x{dt}_{b}")
            for b in range(B) for dt in range(DT)
        }

        # DMA order matters: the sync queue serializes transfers, so emit
        # exactly what the critical path needs first.
        nc.sync.dma_start(w_sb["q"][:], w_in["q"][:])
        nc.sync.dma_start(w_sb["k"][:], w_in["k"][:])
        for dt in range(DT):
            nc.sync.dma_start(xx[(dt, 0)][:], xT[dt * P:(dt + 1) * P, 0:S])
        nc.sync.dma_start(mk[:], mkT[:])
        nc.sync.dma_start(b_sb[:], bqkv[:])
        # (wv and batch-1 x DMAs are emitted after the chunk-0 swap DMAs so
        # the sync queue serves the first scores pair first)

        # Persistent per-core activations.  q_sb/k_sb: [dq, tokens] with
        # head h on partitions [64h, 64h+64).  q_swap/k_swap: partition
        # halves swapped, so each head is also available in the *other*
        # half of the PE array for the kt-pair row-tiled scores.
        q_sb = [qk.tile([P, S], BF16, tag=f"qs{b}", name=f"qs{b}")
                for b in range(B)]
        k_sb = [qk.tile([P, S], BF16, tag=f"ks{b}", name=f"ks{b}")
                for b in range(B)]
        q_sw = [qk.tile([P, S], BF16, tag=f"qw{b}", name=f"qw{b}")
                for b in range(B)]
        k_sw = [qk.tile([P, S], BF16, tag=f"kw{b}", name=f"kw{b}")
                for b in range(B)]
        v_t = [qk.tile([P, S], BF16, tag=f"vt{b}", name=f"vt{b}")
               for b in range(B)]
        vv = [qk.tile([P, KT * W130], BF16, tag=f"vv{b}", name=f"vv{b}")
              for b in range(B)]

        # ones columns of the V_aug layout, written as plain 2D slices so
        # the dependency tracker reliably orders them against ctx reads
        for b in range(B):
            for kt in range(KT):
                for g in range(2):
                    col = kt * W130 + g * W65 + HD
                    _reg("ones", (b, kt, g),
                         nc.vector.memset(vv[b][:, col:col + 1], 1.0))

        vv_ready = [0, 0]   # kts of vv[b] whose copies have been emitted

        # ---------------- projection building blocks ----------------
        BCOL = {"q": 0, "k": 1}

        def evict_proj(pr, b, c, ps):
            c0 = c * CH
            if pr == "v":
                nc.vector.tensor_copy(v_t[b][:, c0:c0 + CH], ps[:])
            else:
                dst = q_sb[b] if pr == "q" else k_sb[b]
                j = BCOL[pr]
                nc.vector.tensor_scalar_add(
                    dst[:, c0:c0 + CH], ps[:], b_sb[:, j:j + 1])

        def proj_granules(pr, b, c):
            """One [128,1024] proj chunk as 4 thunks of 2 dt-steps each."""
            state = {}
            c0 = c * CH

            def gran(g, pr=pr, b=b, c=c, c0=c0, state=state):
                if g == 0:
                    state["ps"] = pj.tile([P, CH], F32, tag="pp", name="pp")
                ps = state["ps"]
                for dt in (2 * g, 2 * g + 1):
                    w = w_sb[pr][:, dt * DC:(dt + 1) * DC]
                    x = xx[(dt, b)]
                    nc.tensor.matmul(ps[:, 0:512], w, x[:, c0:c0 + 512],
                                     start=(dt == 0), stop=(dt == DT - 1))
                    nc.tensor.matmul(ps[:, 512:CH], w, x[:, c0 + 512:c0 + CH],
                                     start=(dt == 0), stop=(dt == DT - 1))
                if g == 3:
                    evict_proj(pr, b, c, ps)
            return [lambda g=g: gran(g) for g in range(4)]

        def emit_swap(pr, b, c):
            """Half-swapped q/k chunk via two SBUF->SBUF DMAs (partition
            re-basing is free on the DMA engines; the sync queue is idle
            mid-attention)."""
            srct = q_sb[b] if pr == "q" else k_sb[b]
            dst = q_sw[b] if pr == "q" else k_sw[b]
            c0 = c * CH
            nc.sync.dma_start(dst[0:HD, c0:c0 + CH],
                              srct[HD:P, c0:c0 + CH])
            nc.sync.dma_start(dst[HD:P, c0:c0 + CH],
                              srct[0:HD, c0:c0 + CH])

        def transp_granules(b, c):
            """8 V^T key-tile transposes into the vv layout, as 2 thunks.

            tp is bf16 [P, 2*CH] so its byte size matches the f32 [P, CH]
            tiles sharing the "pp" slot; only the first CH columns are used.
            """
            state = {}

            def gran(g, b=b, c=c, state=state):
                if g == 0:
                    state["tp"] = pj.tile([P, 2 * CH], BF16, tag="pp",
                                          name="tp")
                tp = state["tp"]
                for j in range(4 * g, 4 * g + 4):
                    kt = c * 8 + j
                    nc.tensor.transpose(
                        tp[:, j * P:(j + 1) * P],
                        v_t[b][:, kt * P:(kt + 1) * P],
                        ident[:],
                    )
                # plain 2D copies per (tile, head): robustly tracked deps
                for j in range(4 * g, 4 * g + 4):
                    kt = c * 8 + j
                    for gh in range(2):
                        r = nc.vector.tensor_copy(
                            vv[b][:, kt * W130 + gh * W65:
                                  kt * W130 + gh * W65 + HD],
                            tp[:, j * P + gh * HD:j * P + gh * HD + HD],
                        )
                        _reg("vvw", (b, kt, gh), r)
                # emission-order progress marker: ctx for these key tiles
                # may only be EMITTED after this point (the dependency
                # tracker is emission-ordered; a read emitted before its
                # writer gets no RAW edge and races)
                vv_ready[b] = max(vv_ready[b], c * 8 + 4 * (g + 1))
            return [lambda g=g: gran(g) for g in range(2)]

        # ---- batch-0 critical path: q-c0/k-c0/k-c1 chase the x DMA ----
        psq0 = sp.tile([P, CH], F32, tag="sps", name="sps")
        psk0 = sp.tile([P, CH], F32, tag="sps", name="sps")
        psk1 = pj.tile([P, CH], F32, tag="pp", name="pp")
        # HAM warm-up: ~4.3us of gapless dummy matmuls on the identity
        # const while the PE waits for the first x tiles; psq0 is
        # overwritten by the chase's start=True accumulation
        for _ in range(10):
            nc.tensor.matmul(psq0[:, 0:512], ident[:], q_sb[0][:, 0:512],
                             start=True, stop=True)
        for dt in range(DT):
            wq = w_sb["q"][:, dt * DC:(dt + 1) * DC]
            wk = w_sb["k"][:, dt * DC:(dt + 1) * DC]
            x = xx[(dt, 0)]
            st, fin = (dt == 0), (dt == DT - 1)
            nc.tensor.matmul(psq0[:, 0:512], wq, x[:, 0:512],
                             start=st, stop=fin)
            nc.tensor.matmul(psq0[:, 512:CH], wq, x[:, 512:CH],
                             start=st, stop=fin)
            nc.tensor.matmul(psk0[:, 0:512], wk, x[:, 0:512],
                             start=st, stop=fin)
            nc.tensor.matmul(psk0[:, 512:CH], wk, x[:, 512:CH],
                             start=st, stop=fin)
            nc.tensor.matmul(psk1[:, 0:512], wk, x[:, CH:CH + 512],
                             start=st, stop=fin)
            nc.tensor.matmul(psk1[:, 512:CH], wk, x[:, CH + 512:S],
                             start=st, stop=fin)
        evict_proj("k", 0, 0, psk0)
        evict_proj("q", 0, 0, psq0)
        evict_proj("k", 0, 1, psk1)
        # chunk-0 swaps feed the first scores pair
        emit_swap("k", 0, 0)
        emit_swap("q", 0, 0)
        nc.sync.dma_start(w_sb["v"][:], w_in["v"][:])
        for dt in range(DT):
            nc.sync.dma_start(xx[(dt, 1)][:], xT[dt * P:(dt + 1) * P, S:BS])

        # ---- deferred work queue, paced into the attention pair loop ----
        G = []
        G += [lambda: emit_swap("k", 0, 1)]
        G += proj_granules("v", 0, 0)
        G += transp_granules(0, 0)
        G += proj_granules("q", 0, 1)
        G += [lambda: emit_swap("q", 0, 1)]
        G += proj_granules("v", 0, 1)
        G += transp_granules(0, 1)
        for c in range(NC_CH):
            G += proj_granules("q", 1, c)
            G += [lambda c=c: emit_swap("q", 1, c)]
            G += proj_granules("k", 1, c)
            G += [lambda c=c: emit_swap("k", 1, c)]
            G += proj_granules("v", 1, c)
            G += transp_granules(1, c)
        fi = 0
        if not FILLER_INTERLEAVE:
            for item in G:
                item()
            fi = len(G)

        # ---------------- attention ----------------
        with nc.named_scope("attn"):
            n_emitted_pairs = 0
            for u in range(NU):
                b, h = u // HPC, u % HPC
                # head h's rows in the natural / swapped layouts
                nat = slice(h * HD, (h + 1) * HD)
                swp = slice((1 - h) * HD, (2 - h) * HD)
                for l in range(NL):
                    q0 = l * QH
                    cps = cp.tile([W65, QH], F32, tag="cps", name="cps")
                    pts = [None] * KT

                    def emit_ctx(kt, b=b, h=h, cps=cps, pts=pts):
                        o0 = kt * W130 + h * W65
                        va = vv[b][:, o0:o0 + W65]
                        _reg("ctx", (u, l, kt, 0), nc.tensor.matmul(
                            cps[:, 0:512], va, pts[kt][:, 0:512],
                            start=(kt == 0), stop=(kt == KT - 1)))
                        _reg("ctx", (u, l, kt, 1), nc.tensor.matmul(
                            cps[:, 512:QH], va, pts[kt][:, 512:QH],
                            start=(kt == 0), stop=(kt == KT - 1)))

                    pending = []  # kts with pts awaiting ctx emission

                    def drain_until_vv(kt, b=b):
                        nonlocal fi
                        while vv_ready[b] <= kt and fi < len(G):
                            G[fi]()
                            fi += 1
                        assert vv_ready[b] > kt, "granule queue exhausted"

                    def flush_ctx(force_upto=None):
                        while pending and (
                                pending[0] < vv_ready[b]
                                or (force_upto is not None
                                    and pending[0] <= force_upto)):
                            kt = pending.pop(0)
                            if kt >= vv_ready[b]:
                                drain_until_vv(kt)
                            emit_ctx(kt)

                    for p in range(KT // 2):
                        ktE, ktO = 2 * p, 2 * p + 1
                        # pt-ring safety: slots reused by this pair's exps
                        # must have their reader ctx emitted first
                        flush_ctx(force_upto=ktO - 8)
                        sE = sp.tile([P, QH], F32, tag="sps", name="sps")
                        sO = sp.tile([P, QH], F32, tag="sps", name="sps")
                        kE = k_sb[b][nat, ktE * P:(ktE + 1) * P]
                        kO = k_sw[b][swp, ktO * P:(ktO + 1) * P]
                        qE = q_sb[b][nat, q0:q0 + QH]
                        qO = q_sw[b][swp, q0:q0 + QH]
                        # interleave E/O so the two K=64 row-tiles can run
                        # concurrently when the PE is the constraint
                        nc.tensor.matmul(sE[:, 0:512], kE, qE[:, 0:512],
                                         start=True, stop=True)
                        nc.tensor.matmul(sO[:, 0:512], kO, qO[:, 0:512],
                                         start=True, stop=True)
                        nc.tensor.matmul(sE[:, 512:QH], kE, qE[:, 512:QH],
                                         start=True, stop=True)
                        nc.tensor.matmul(sO[:, 512:QH], kO, qO[:, 512:QH],
                                         start=True, stop=True)
                        flush_ctx()
                        for kt, s in ((ktE, sE), (ktO, sO)):
                            pt = ptp.tile([P, QH], BF16, tag="pt")
                            r = nc.scalar.activation(
                                pt[:], s[:],
                                mybir.ActivationFunctionType.Exp,
                                bias=mk[:, b * KT + kt:b * KT + kt + 1],
                                scale=SCALE,
                            )
                            _reg("exp", (u, l, kt), r)
                            pts[kt] = pt
                        pending += [ktE, ktO]
                        # deferred proj/layout granules ride the PE slack
                        n_emitted_pairs += 1
                        budget = 3 if n_emitted_pairs <= 8 else 2
                        while budget > 0 and fi < len(G):
                            G[fi]()
                            fi += 1
                            budget -= 1
                    flush_ctx(force_upto=KT - 1)
                    ob = obp.tile([W65, QH], F32, tag="ob")
                    nc.vector.tensor_copy(ob[:], cps[:])
                    nc.sync.dma_start(out[u, :, q0:q0 + QH], ob[:])
            while fi < len(G):
                G[fi]()
                fi += 1

    if DEDUPE_LDWEIGHTS:
        _dedupe_ldweights(nc)
    nc.compile()
    return nc


def _prep_in_maps(hidden_states, attention_mask, Wq, bq, Wk, bk, Wv, bv):
    bf = ml_dtypes.bfloat16
    hs = np.asarray(hidden_states, dtype=np.float32).reshape(BS, D)
    xT = np.ascontiguousarray(hs.T).astype(bf)
    # mask pre-tiled: mkT[p, b*KT + t] = mask[b, t*P + p]
    mkT = np.ascontiguousarray(
        np.asarray(attention_mask, dtype=np.float32).reshape(B, KT, P)
        .transpose(2, 0, 1).reshape(P, B * KT)
    )
    Ws = {"q": np.asarray(Wq, np.float32), "k": np.asarray(Wk, np.float32),
          "v": np.asarray(Wv, np.float32)}
    bs = {"q": np.asarray(bq, np.float32), "k": np.asarray(bk, np.float32),
          "v": np.asarray(bv, np.float32)}
    in_maps = []
    for c in range(N_CORES):
        sl = slice(c * DC, (c + 1) * DC)
        m = {"xT": xT, "mkT": mkT}
        for pr in "qkv":
            # pre-tiled: [P, DT*DC], column block dt = rows [dt*P,(dt+1)*P)
            wc = Ws[pr][:, sl].reshape(DT, P, DC).transpose(1, 0, 2)
            m[f"w{pr}"] = np.ascontiguousarray(wc.reshape(P, DT * DC)).astype(bf)
        m["bqkv"] = np.ascontiguousarray(
            np.stack([bs["q"][sl], bs["k"][sl], bs["v"][sl]], axis=1)
        )
        in_maps.append(m)
    return in_maps


def _gather(results, bv):
    bv = np.asarray(bv, np.float32)
    full = np.empty((B, S, D), dtype=np.float32)
    for c in range(N_CORES):
        o = results[c]["out"]  # [NU, 65, S] unnormalized ctx^T + denom row
        for b in range(B):
            for hl in range(HPC):
                u = b * HPC + hl
                col = c * DC + hl * HD
                ctx = o[u, :HD, :] / o[u, HD:HD + 1, :]
                full[b, :, col:col + HD] = ctx.T + bv[col:col + HD]
    return full


def kernel(hidden_states, attention_mask, Wq, bq, Wk, bk, Wv, bv, **run_kwargs):
    global _cached_nc
    if _cached_nc is None:
        _cached_nc = build_nc()
    in_maps = _prep_in_maps(
        hidden_states, attention_mask, Wq, bq, Wk, bk, Wv, bv
    )
    res = run_bass_kernel_spmd(
        _cached_nc, in_maps, core_ids=list(range(N_CORES)), **run_kwargs
    )
    full = _gather(res.results, bv)
    if run_kwargs:
        kernel.last_result = res
    return full
